# revision 26
# baseline (speedup 1.0000x reference)
"""Trainium2 Bass kernel for nn_ExtSummModel (extractive summarization).

Data-parallel over docs: 8 cores x 4 docs, single SPMD launch.

Per-core program:
  - pipelined production: per 32-sentence block, embedding gather (per-word
    indirect DMA, 4 parallel accumulation chains) -> sum -> PE transpose ->
    fp32r input projection (bias and 1/L folded into the weights) -> giT
  - merged bidirectional GRU: one fused instruction stream computes the f
    and b cell updates per iteration (b runs time-reversed); production of
    later blocks and the srep DRAM bounce are interleaved into the loop
  - tail: hT AllGather (overlapped with the topic phase), topic
    representation, two attention branches, dense + logits; big matmuls
    in fp32r (1 cycle/row at >=256 moving elements vs 4 for fp32).

Self-contained: hardcodes shapes; host side only shards/packs numpy inputs.
"""
import sys

sys.path.insert(0, "/opt/trn_rl_repo")

import numpy as np
import concourse.bacc as bacc
import concourse.bass as bass
import concourse.mybir as mybir
import concourse.tile as tile
from concourse.bass_utils import run_bass_kernel_spmd
from concourse.masks import make_identity

B, S, L, E, H, T, V, D = 32, 256, 24, 300, 256, 16, 50000, 256
NC = 8
BD = B // NC          # 4 docs per core
SP = S + 2            # padded sentence axis
H2, H3, H4 = 2 * H, 3 * H, 4 * H
NBLK = 8              # sentence blocks of 32 s (128 (s,d) columns)
SD = S * BD           # 1024 (s,d) columns
GI = S * 24           # gi columns per direction
EKC = [128, 128, 44]  # wihT chunks

f32 = mybir.dt.float32
f32r = mybir.dt.float32r
i32 = mybir.dt.int32
AF = mybir.ActivationFunctionType
OP = mybir.AluOpType

_BUILT = None


def _emit(tc, nc, ein, logits, dbg):
    from contextlib import ExitStack
    ctx = ExitStack()
    res = ctx.enter_context(tc.tile_pool(name="res", bufs=1))
    dram = ctx.enter_context(tc.tile_pool(name="dram", bufs=1, space="DRAM"))

    # ---------------- residents ----------------
    from contextlib import ExitStack as _ES
    bctx = ExitStack()
    gw = bctx.enter_context(tc.tile_pool(name="gw", bufs=1))

    ident = res.tile([128, 128], f32, tag="ident", name="ident")
    make_identity(nc, ident[:])
    identr = res.tile([128, 128], f32r, tag="identr", name="identr")
    nc.vector.tensor_copy(identr[:], ident[:])

    weight_loads = []
    wf = [gw.tile([128, H3], f32r, tag=f"wf{k}", name=f"wf{k}") for k in range(2)]
    wb = [gw.tile([128, H3], f32r, tag=f"wb{k}", name=f"wb{k}") for k in range(2)]
    for k in range(2):
        weight_loads.append((wf[k], ein["whhT_f"].ap()[k * 128:(k + 1) * 128, :]))
        weight_loads.append((wb[k], ein["whhT_b"].ap()[k * 128:(k + 1) * 128, :]))
    wdir = {0: wf, 1: wb}

    wih = {}
    for di, d in enumerate(("f", "b")):
        for k in range(3):
            t_ = gw.tile([EKC[k], H3], f32r, tag=f"wih{d}{k}", name=f"wih{d}{k}")
            weight_loads.append((t_, ein[f"wihT_{d}"].ap()[128 * k:128 * k + EKC[k], :]))
            wih[(di, k)] = t_

    bhhn = {}
    brow = {}
    for di, d in enumerate(("f", "b")):
        r = gw.tile([1, H], f32r, tag=f"bhhn{d}", name=f"bhhn{d}")
        weight_loads.append((r, ein[f"bhhn_{d}"].ap()))
        bhhn[di] = r
        br = gw.tile([1, H3], f32r, tag=f"brow{d}", name=f"brow{d}")
        weight_loads.append((br, ein[f"brow_{d}"].ap()))
        brow[di] = br

    vatt = res.tile([128, 8], f32r, tag="vatt", name="vatt")
    nc.sync.dma_start(vatt[:], ein["v_att"].ap().rearrange("(m p) o -> p (m o)", p=128))
    wout = res.tile([128, 2], f32r, tag="wout", name="wout")
    nc.sync.dma_start(wout[:], ein["w_out"].ap().rearrange("(m p) o -> p (m o)", p=128))
    bdna = res.tile([128, 2], f32, tag="bdna", name="bdna")
    nc.sync.dma_start(bdna[:], ein["b_dna"].ap().rearrange("(m p) -> p m", p=128))
    bout = res.tile([1, 1], f32, tag="bout", name="bout")
    nc.sync.dma_start(bout[:], ein["b_out"].ap().rearrange("(a o) -> a o", a=1))
    onesf = res.tile([1, 512], f32, tag="onesf", name="onesf")
    nc.gpsimd.memset(onesf[:], 1.0)
    onesr = res.tile([1, 512], f32r, tag="onesr", name="onesr")
    nc.vector.tensor_copy(onesr[:], onesf[:])

    wid_sb = gw.tile([128, NBLK * L], i32, tag="wid", name="wid")
    nc.sync.dma_start(wid_sb[:], ein["wid"].ap())

    # big seq-state residents
    srepT = res.tile([128, 4 * SD], f32r, tag="srepT", name="srepT")
    giT = gw.tile([128, 2 * GI], f32r, tag="giT", name="giT")
    xT = [gw.tile([EKC[k], SD], f32r, tag=f"xT{k}", name=f"xT{k}") for k in range(3)]

    srep = dram.tile([BD * SP, H2], f32, name="srep")
    zrow = res.tile([1, H2], f32, tag="zrow", name="zrow")
    nc.gpsimd.memset(zrow[:], 0.0)
    for d0 in range(BD):
        nc.sync.dma_start(srep[d0 * SP:d0 * SP + 1, :], zrow[:])
        nc.sync.dma_start(srep[d0 * SP + SP - 1:d0 * SP + SP, :], zrow[:])

    zct = res.tile([128, 32], f32, tag="zct", name="zct")
    nc.gpsimd.memset(zct[:], 0.0)

    # giT as [p, group(s,dir), m(6), d(4)]
    gi4 = giT[:].rearrange("p (g m c) -> p g m c", m=6, c=4)
    # srepT as [p, k(4), c(SD)]
    sr3 = srepT[:].rearrange("p (k c) -> p k c", c=SD)

    pb = bctx.enter_context(tc.tile_pool(name="pb", bufs=2))
    pbp = bctx.enter_context(tc.tile_pool(name="pbp", bufs=2, space="PSUM"))
    pbt = bctx.enter_context(tc.tile_pool(name="pbt", bufs=2, space="PSUM"))
    g_sb = bctx.enter_context(tc.tile_pool(name="gsb", bufs=3))
    g_ps = bctx.enter_context(tc.tile_pool(name="gps", bufs=2, space="PSUM"))

    # ---------------- production (phase B) ----------------
    xp_cache = {}

    def get_xp(bl, j):
        key = (bl, j)
        if key not in xp_cache:
            xp_cache[key] = pb.tile([128, E], f32, tag=f"xp{j}", name=f"xp{j}")
        return xp_cache[key]

    def emit_gather(bl, l):
        xp = get_xp(bl, l // 6)
        nc.gpsimd.indirect_dma_start(
            out=xp[:], out_offset=None, in_=ein["emb"].ap(),
            in_offset=bass.IndirectOffsetOnAxis(
                ap=wid_sb[:, bl * L + l:bl * L + l + 1], axis=0),
            compute_op=(OP.bypass if l % 6 == 0 else OP.add))

    def emit_add(bl, which):
        ps = [get_xp(bl, j) for j in range(4)]
        if which == 0:
            nc.vector.tensor_add(ps[0][:], ps[0][:], ps[1][:])
        elif which == 1:
            nc.vector.tensor_add(ps[2][:], ps[2][:], ps[3][:])
        else:
            nc.vector.tensor_add(ps[0][:], ps[0][:], ps[2][:])

    def emit_transpose(bl, k, eng):
        xs = get_xp(bl, 0)
        ps = pbt.tile([128, 128], f32, tag="tp", name="tp")
        nk = EKC[k]
        nc.tensor.transpose(ps[0:nk, :], xs[:, 128 * k:128 * k + nk], ident[:])
        dst = xT[k][0:nk, bl * 128:(bl + 1) * 128]
        if eng == "v":
            nc.vector.tensor_copy(dst, ps[0:nk, :])
        else:
            nc.scalar.activation(dst, ps[0:nk, :], AF.Copy)

    def emit_proj(pair, di, m, eng):
        b0, b1 = pair
        step = b1 - b0
        ps = pbp.tile([128, 256], f32, tag="projp", name="projp")
        for k in range(3):
            rhs = xT[k][:].rearrange("p (blk c) -> p blk c", c=128)[:, b0:b1 + 1:step, :]
            nc.tensor.matmul(ps[:], wih[(di, k)][:, m * 128:(m + 1) * 128],
                             rhs, start=(k == 0), stop=False)
        nc.tensor.matmul(ps[:], brow[di][:, m * 128:(m + 1) * 128],
                         onesr[:, 0:256], start=False, stop=True)
        src = ps[:].rearrange("p (blk s d) -> p blk s d", blk=2, d=4)
        for bi, bl in enumerate((b0, b1)):
            g0 = di * 256 + bl * 32
            dv = gi4[:, g0:g0 + 32, m, :]
            if eng == "v":
                nc.vector.tensor_copy(dv, src[:, bi, :, :])
            else:
                nc.scalar.activation(dv, src[:, bi, :, :], AF.Copy)

    def emit_bounce(k4, sb, eng):
        ps2 = pbt.tile([128, 128], f32, tag="tp", name="tp")
        nc.tensor.transpose(
            ps2[:],
            srepT[:, k4 * SD + sb * 128:k4 * SD + (sb + 1) * 128].bitcast(f32),
            ident[:])
        st2 = pb.tile([128, 128], f32, tag="s2st", name="s2st")
        if eng == "v":
            nc.vector.tensor_copy(st2[:], ps2[:])
        else:
            nc.scalar.activation(st2[:], ps2[:], AF.Copy)
        half = 0 if k4 < 2 else 1
        kk = k4 % 2
        dst = srep[:].rearrange("(d sp) c -> sp d c", d=BD)[
            1 + sb * 32: 1 + (sb + 1) * 32, :,
            kk * 128 + half * H: kk * 128 + half * H + 128]
        nc.sync.dma_start(dst, st2[:])

    # ---------------- production schedule ----------------
    sched = {}

    def at(it, fn, *a, **kw):
        sched.setdefault(max(0, min(it, 255)), []).append((fn, a, kw))

    GORDER = [j * 6 + i for i in range(6) for j in range(4)]  # chain-rotating

    def emit_block_gathers(bl):
        for l in GORDER:
            emit_gather(bl, l)

    def emit_block_sum_transpose(bl, eng):
        for w in range(3):
            emit_add(bl, w)
        for k in range(3):
            emit_transpose(bl, k, eng)

    # head: pair (0,7) fully; then queue gathers for pair (1,6)
    emit_block_gathers(0)
    emit_block_gathers(7)
    for t_, ap_ in weight_loads:
        nc.sync.dma_start(t_[:], ap_)
    emit_block_sum_transpose(0, "v")
    emit_block_sum_transpose(7, "v")
    for di in range(2):
        for m in range(6):
            emit_proj((0, 7), di, m, "v")
    emit_block_gathers(1)
    emit_block_gathers(6)

    # pair (2,5) gathers: 2 per iter from iter 1; pair (3,4) from iter 25
    it = 1
    for bl in (2, 5):
        for i, l in enumerate(GORDER):
            at(it, emit_gather, bl, l)
            if i % 2 == 1:
                it += 1
    it = 25
    for bl in (3, 4):
        for i, l in enumerate(GORDER):
            at(it, emit_gather, bl, l)
            if i % 2 == 1:
                it += 1

    def sched_pair_tail(pair, t0):
        b0, b1 = pair
        deadline = 32 * b0 - 1          # all proj emitted strictly before this
        for w in range(3):
            at(t0 + w, emit_add, b0, w)
            at(t0 + 1 + w, emit_add, b1, w)
        for k in range(3):
            at(t0 + 3 + k, emit_transpose, b0, k, "v" if k % 2 else "a")
            at(t0 + 4 + k, emit_transpose, b1, k, "a" if k % 2 else "v")
        i = 0
        for di in range(2):
            for m in range(6):
                it_p = min(t0 + 6 + i // 2, deadline)
                assert it_p <= deadline
                at(it_p, emit_proj, pair, di, m, "a" if i % 2 else "v")
                i += 1

    sched_pair_tail((1, 6), 20)
    sched_pair_tail((2, 5), 50)
    sched_pair_tail((3, 4), 78)

    # srep bounce: chunk sb of f and chunk (7-sb) of b complete after iter 32(sb+1)-1
    for sb in range(8):
        base = 32 * (sb + 1)
        at(base + 1, emit_bounce, 0, sb, "v")
        at(base + 2, emit_bounce, 1, sb, "a")
        at(base + 3, emit_bounce, 2, 7 - sb, "v")
        at(base + 4, emit_bounce, 3, 7 - sb, "a")

    # ---------------- GRU (merged f/b) ----------------
    sc2_prev = None
    for t in range(S):
        tf, tb = t, S - 1 - t
        tdir = {0: tf, 1: tb}
        rz = g_ps.tile([128, 32], f32, tag="rz", name="rz")
        nps = g_ps.tile([128, 16], f32, tag="nps", name="nps")
        # --- PE: gate matmuls ---
        for di in range(2):
            td = tdir[di]
            if t == 0:
                for m in range(4):
                    nc.tensor.matmul(rz[:, di * 16 + m * 4:di * 16 + (m + 1) * 4],
                                     identr[:], gi4[:, di * 256 + td, m, :],
                                     start=True, stop=True)
                for m in range(2):
                    nc.tensor.matmul(nps[:, di * 8 + m * 4:di * 8 + (m + 1) * 4],
                                     bhhn[di][:, m * 128:(m + 1) * 128],
                                     onesr[:, 0:BD], start=True, stop=True)
            else:
                # seed psum with gi (rz) and bhh_n (n), then accumulate h-mms
                nc.tensor.matmul(rz[:, di * 16:di * 16 + 16], identr[:],
                                 gi4[:, di * 256 + td, 0:4, :].rearrange(
                                     "p m c -> p (m c)"),
                                 start=True, stop=False)
                nps8 = nps[:, di * 8:di * 8 + 8].rearrange(
                    "p (m c) -> p m c", c=4)
                for m in range(2):
                    nc.tensor.matmul(nps8[:, m, :],
                                     bhhn[di][:, m * 128:(m + 1) * 128],
                                     onesr[:, 0:BD], start=True, stop=False)
                for m in range(4):
                    o = rz[:, di * 16 + m * 4:di * 16 + (m + 1) * 4]
                    for k in range(2):
                        q0 = (di * 2 + k) * 4
                        hprev = sc2_prev[:].rearrange(
                            "p (q two) -> p q two", two=2)[:, q0:q0 + 4, 1]
                        nc.tensor.matmul(o, wdir[di][k][:, m * 128:(m + 1) * 128],
                                         hprev, start=False, stop=(k == 1))
                for m in range(2):
                    o = nps[:, di * 8 + m * 4:di * 8 + (m + 1) * 4]
                    for k in range(2):
                        q0 = (di * 2 + k) * 4
                        hprev = sc2_prev[:].rearrange(
                            "p (q two) -> p q two", two=2)[:, q0:q0 + 4, 1]
                        nc.tensor.matmul(o, wdir[di][k][:, (4 + m) * 128:(5 + m) * 128],
                                         hprev, start=False, stop=(k == 1))

        # --- Act: sigmoid(r,z) and 1-z = sigmoid(-pre_z) ---
        rzs = g_sb.tile([128, 32], f32, tag="rzs", name="rzs")
        nc.scalar.activation(rzs[:], rz[:], AF.Sigmoid)
        # --- DVE: n path; z*h_prev in parallel ---
        rv = rzs[:].rearrange("p (dir g) -> p dir g", dir=2)[:, :, 0:8]
        rn = g_sb.tile([128, 16], f32, tag="rn", name="rn")
        rnv = rn[:].rearrange("p (dir g) -> p dir g", dir=2)
        nc.vector.tensor_mul(rnv, rv, nps[:].rearrange("p (dir g) -> p dir g", dir=2))
        npre = g_sb.tile([128, 16], f32, tag="npre", name="npre")
        nprev4 = npre[:].rearrange("p (dir m c) -> p dir m c", dir=2, c=4)
        rnv4 = rn[:].rearrange("p (dir m c) -> p dir m c", dir=2, c=4)
        gstep = 256 + tb - tf
        gin4 = gi4[:, tf:256 + tb + 1:gstep, 4:6, :]
        nc.vector.tensor_add(nprev4, rnv4, gin4)

        ntzh = g_sb.tile([128, 32], f32, tag="ntzh", name="ntzh")
        n5 = ntzh[:].rearrange("p (dir k d two) -> p dir k d two", dir=2, k=2, two=2)
        if t > 0:
            zv = rzs[:].rearrange("p (dir k d) -> p dir k d", dir=2, d=4)[:, :, 2:4, :]
            s5 = sc2_prev[:].rearrange("p (dir k d two) -> p dir k d two",
                                       dir=2, k=2, two=2)
            for di in range(2):
                nc.vector.tensor_mul(n5[:, di, :, :, 1], zv[:, di, :, :],
                                     s5[:, di, :, :, 1])

        zps4 = rz[:].rearrange("p (dir k d) -> p dir k d", dir=2, d=4)[:, :, 2:4, :]
        z5 = zct[:].rearrange("p (dir k d two) -> p dir k d two", dir=2, k=2, two=2)
        nc.scalar.activation(z5[:, :, :, :, 1], zps4, AF.Sigmoid, scale=-1.0)

        # --- Act: tanh -> ntzh even cols ---
        nc.scalar.activation(n5[:, :, :, :, 0].rearrange("p dir k d -> p (dir k d)"),
                             npre[:], AF.Tanh)

        # --- DVE: scan computes h2 = (1-z)*nt + z*h at odd cols ---
        sc2 = g_sb.tile([128, 32], f32r, tag="sc2", name="sc2")
        if t == 0:
            # no zh term: h2 = (1-z)*nt; seed ntzh odd with zeros via scan of
            # d1 odd = 0: write zeros into odd cols once
            nc.vector.tensor_scalar_mul(
                n5[:, :, :, :, 1].rearrange("p dir k d -> p (dir k d)"),
                n5[:, :, :, :, 0].rearrange("p dir k d -> p (dir k d)"), 0.0)
        nc.vector.tensor_tensor_scan(sc2[:], zct[:], ntzh[:], 0.0,
                                     op0=OP.mult, op1=OP.add)
        # off-path: persist h2 into srepT for attention/bounce/collective
        s5o = sc2[:].rearrange("p (dir k d two) -> p dir k d two",
                               dir=2, k=2, two=2)
        for di in range(2):
            td = tdir[di]
            dst = sr3[:, di * 2:di * 2 + 2, td * 4:td * 4 + 4]
            nc.vector.tensor_copy(dst, s5o[:, di, :, :, 1])
        sc2_prev = sc2

        import os as _os
        _PRIO = int(_os.environ.get("KPRIO", "0"))
        if _PRIO:
            with tc.high_priority(offset=_PRIO):
                for fn, a, kw in sched.get(t, ()):
                    fn(*a, **kw)
        else:
            for fn, a, kw in sched.get(t, ()):
                fn(*a, **kw)

    if dbg:
        nc.sync.dma_start(dbg["d_xT0"].ap(), xT[0][:].bitcast(f32))
        nc.sync.dma_start(dbg["d_gif"].ap(), giT[:, 0:GI].bitcast(f32))
        nc.sync.dma_start(dbg["d_srepT"].ap(), srepT[:].bitcast(f32))

    # ---------------- hT extraction + collective ----------------
    cc_in = dram.tile([8, H], f32, name="cc_in")
    cc_out = dram.tile([8 * NC, H], f32, name="cc_out")
    for di in range(2):
        t0 = 255 if di == 0 else 0
        for k in range(2):
            iv = sr3[:, di * 2 + k, t0 * 4:t0 * 4 + 4]          # [p, d]
            ov = cc_in[di * 4:(di + 1) * 4,
                       k * 128:(k + 1) * 128].rearrange("d p -> p d")
            nc.sync.dma_start(ov, iv.bitcast(f32))
    nc.gpsimd.collective_compute(
        "AllGather", OP.bypass, replica_groups=[list(range(NC))],
        ins=[cc_in.opt()], outs=[cc_out.opt()])

    bctx.close()

    # ---------------- topic phase (overlaps collective) ----------------
    topicrepT = [res.tile([128, SD], f32r, tag=f"trep{c}", name=f"trep{c}")
                 for c in range(4)]
    with (
        tc.tile_pool(name="p5w", bufs=2) as p5w,
        tc.tile_pool(name="p5s", bufs=1) as p5s,
        tc.tile_pool(name="p5p", bufs=2, space="PSUM") as p5p,
    ):
        iota_i = p5w.tile([32, S], i32, tag="iotai", name="iotai")
        nc.gpsimd.iota(iota_i[:], pattern=[[1, S]], base=0, channel_multiplier=0)
        iota_f = p5w.tile([32, S], f32, tag="iotaf", name="iotaf")
        nc.vector.tensor_copy(iota_f[:], iota_i[:])
        pidx = p5w.tile([32, 1], i32, tag="pidx", name="pidx")
        nc.gpsimd.iota(pidx[:], pattern=[[0, 1]], base=0, channel_multiplier=1)
        big15 = p5w.tile([32, 1], f32, tag="big15", name="big15")
        nc.vector.tensor_scalar(big15[:], pidx[:], T - 1, 1.0e9,
                                op0=OP.is_equal, op1=OP.mult)
        dspi = p5w.tile([32, BD], i32, tag="dspi", name="dspi")
        nc.gpsimd.iota(dspi[:], pattern=[[SP, BD]], base=0, channel_multiplier=0)

        tse_sb = p5s.tile([32, 2 * BD], i32, tag="tse", name="tse")
        nc.gpsimd.memset(tse_sb[:], 0)
        nc.sync.dma_start(
            tse_sb[0:T, :].rearrange("t (d two) -> t d two", two=2),
            ein["tse"].ap().rearrange("d t two -> t d two"))
        tsev = tse_sb[:].rearrange("t (d two) -> t d two", two=2)
        st_i, en_i = tsev[:, :, 0], tsev[:, :, 1]

        off = {}
        for nm in ("en", "sm1", "st", "ep1"):
            off[nm] = p5s.tile([32, BD], i32, tag=f"off{nm}", name=f"off{nm}")
        nc.vector.tensor_tensor(off["en"][:], en_i, dspi[:], op=OP.add)
        nc.vector.tensor_scalar(off["sm1"][:], st_i, -1, 0, op0=OP.add, op1=OP.max)
        nc.vector.tensor_tensor(off["sm1"][:], off["sm1"][:], dspi[:], op=OP.add)
        nc.vector.tensor_tensor(off["st"][:], st_i, dspi[:], op=OP.add)
        nc.vector.tensor_scalar(off["ep1"][:], off["en"][:], 1, None, op0=OP.add)

        g = {}
        for nm in ("en", "sm1", "st", "ep1"):
            gt = p5s.tile([32, BD * H2], f32, tag=f"g{nm}", name=f"g{nm}")
            gv = gt[:].rearrange("t (d c) -> t d c", d=BD)
            for d0 in range(BD):
                nc.gpsimd.indirect_dma_start(
                    out=gv[:, d0, :], out_offset=None, in_=srep[:],
                    in_offset=bass.IndirectOffsetOnAxis(
                        ap=off[nm][:, d0:d0 + 1], axis=0))
            g[nm] = gt
        tm = p5w.tile([32, BD * H2], f32r, tag="tmat", name="tmat")
        tmv = tm[:].rearrange("t (d c) -> t d c", d=BD)
        gw = {nm: g[nm][:].rearrange("t (d c) -> t d c", d=BD) for nm in g}
        nc.vector.tensor_sub(tmv[:, :, 0:H], gw["en"][:, :, 0:H], gw["sm1"][:, :, 0:H])
        nc.vector.tensor_sub(tmv[:, :, H:], gw["st"][:, :, H:], gw["ep1"][:, :, H:])

        enf = p5s.tile([32, BD], f32, tag="enf", name="enf")
        nc.vector.tensor_copy(enf[:], en_i)
        nc.vector.tensor_tensor(enf[:], enf[:], big15[:].to_broadcast([32, BD]),
                                op=OP.add)
        epf = p5s.tile([32, BD], f32, tag="epf", name="epf")
        nc.gpsimd.memset(epf[:], 0.0)
        nc.sync.dma_start(epf[1:T, :], enf[0:T - 1, :])
        ohs = []
        for d0 in range(BD):
            e_m = p5w.tile([32, S], f32, tag="em", name="em")
            nc.vector.tensor_scalar(e_m[:], iota_f[:], enf[:, d0:d0 + 1], None,
                                    op0=OP.is_lt)
            ep_m = p5w.tile([32, S], f32, tag="epm", name="epm")
            nc.vector.tensor_scalar(ep_m[:], iota_f[:], epf[:, d0:d0 + 1], None,
                                    op0=OP.is_lt)
            oh = p5w.tile([32, S], f32r, tag=f"oh{d0}", name=f"oh{d0}")
            nc.vector.tensor_mul(oh[:], e_m[:], ep_m[:])
            nc.vector.tensor_sub(oh[:], e_m[:], oh[:])
            ohs.append(oh)

        for d0 in range(BD):
            for c in range(4):
                ps = p5p.tile([128, S], f32, tag="trp", name="trp")
                nc.tensor.matmul(ps[:], tmv[:, d0, c * 128:(c + 1) * 128],
                                 ohs[d0][:], start=True, stop=True)
                nc.vector.tensor_copy(
                    topicrepT[c][:].rearrange("p (s d) -> p d s", d=BD)[:, d0, :],
                    ps[:])

    if dbg:
        nc.sync.dma_start(dbg["d_trep0"].ap(), topicrepT[0][:].bitcast(f32))

    # ---------------- attention weights + doc path ----------------
    p6r = ctx.enter_context(tc.tile_pool(name="p6r", bufs=1))
    watt = [p6r.tile([128, H4], f32r, tag=f"watt{k}", name=f"watt{k}") for k in range(8)]
    for k in range(8):
        nc.sync.dma_start(watt[k][:], ein["w_att"].ap()[k * 128:(k + 1) * 128, :])
    wdna = [p6r.tile([128, D], f32r, tag=f"wdna{k}", name=f"wdna{k}") for k in range(8)]
    for k in range(8):
        nc.sync.dma_start(wdna[k][:], ein["w_dna"].ap()[k * 128:(k + 1) * 128, :])

    docrepT = [res.tile([128, SD], f32r, tag=f"drep{c}", name=f"drep{c}")
               for c in range(4)]
    with (
        tc.tile_pool(name="p4w", bufs=2) as p4w,
        tc.tile_pool(name="p4p", bufs=2, space="PSUM") as p4p,
    ):
        dvr_sb = p4w.tile([8, 1], i32, tag="dvr", name="dvr")
        nc.sync.dma_start(dvr_sb[:], ein["dvrows"].ap())
        dvraw = p4w.tile([8, H], f32, tag="dvraw", name="dvraw")
        nc.gpsimd.indirect_dma_start(
            out=dvraw[:], out_offset=None, in_=cc_out[:],
            in_offset=bass.IndirectOffsetOnAxis(ap=dvr_sb[:, 0:1], axis=0))
        tps = []
        for half in range(2):
            ps = p4p.tile([128, 8], f32, tag="dvt", name="dvt")
            nc.tensor.transpose(ps[:], dvraw[:, half * 128:(half + 1) * 128],
                                ident[0:8, 0:8])
            tps.append(ps)
        for c in range(4):
            t_ = p4w.tile([128, BD], f32, tag=f"dvT{c}", name=f"dvT{c}")
            src2 = tps[c % 2][:].rearrange("p (b two) -> p two b", two=2)[:, c // 2, :]
            nc.vector.tensor_copy(t_[:], src2)
            for d0 in range(BD):
                nc.vector.tensor_copy(
                    docrepT[c][:].rearrange("p (s d) -> p d s", d=BD)[:, d0, :],
                    t_[:, d0:d0 + 1].to_broadcast([128, S]))

    # ---------------- attention ----------------
    scores_w = {}
    with (
        tc.tile_pool(name="p6w", bufs=2) as p6w,
        tc.tile_pool(name="p6one", bufs=1) as p6one,
        tc.tile_pool(name="p6s", bufs=3) as p6s,
    ):
        with (
            tc.tile_pool(name="p6pa", bufs=4, space="PSUM") as p6pa,
            tc.tile_pool(name="p6ps", bufs=1, space="PSUM") as p6ps,
        ):
            for kind in ("ts", "ds"):
                reps = docrepT if kind == "ds" else topicrepT
                sc_ps = [p6ps.tile([1, 512], f32, tag=f"scp{kind}{nh}",
                                   name=f"scp{kind}{nh}") for nh in range(2)]
                for m in range(8):
                    pm = [p6pa.tile([128, 512], f32, tag="attp", name="attp")
                          for _ in range(2)]
                    for k in range(8):
                        for nh in range(2):
                            rhsap = (reps[k][:, nh * 512:(nh + 1) * 512] if k < 4
                                     else srepT[:, (k - 4) * SD + nh * 512:
                                                (k - 4) * SD + (nh + 1) * 512])
                            nc.tensor.matmul(pm[nh][:],
                                             watt[k][:, m * 128:(m + 1) * 128],
                                             rhsap, start=(k == 0), stop=(k == 7))
                    pt = p6w.tile([128, H4], f32r, tag="ptanh", name="ptanh")
                    for nh in range(2):
                        nc.scalar.activation(pt[:, nh * 512:(nh + 1) * 512],
                                             pm[nh][:], AF.Tanh)
                    for nh in range(2):
                        nc.tensor.matmul(sc_ps[nh][:], vatt[:, m:m + 1],
                                         pt[:, nh * 512:(nh + 1) * 512],
                                         start=(m == 0), stop=(m == 7))
                sc = p6one.tile([1, SD], f32, tag=f"sc{kind}", name=f"sc{kind}")
                for nh in range(2):
                    nc.vector.tensor_copy(sc[:, nh * 512:(nh + 1) * 512], sc_ps[nh][:])
                w_ = p6one.tile([1, SD], f32r, tag=f"w{kind}", name=f"w{kind}")
                for d0 in range(BD):
                    sl = sc[:].rearrange("o (s d) -> o d s", d=BD)[:, d0, :]
                    wl = w_[:].rearrange("o (s d) -> o d s", d=BD)[:, d0, :]
                    mx = p6s.tile([1, 1], f32, tag="mx", name="mx")
                    nc.vector.reduce_max(mx[:], sl, axis=mybir.AxisListType.X)
                    sh = p6s.tile([1, S], f32, tag="sh", name="sh")
                    nc.vector.tensor_scalar(sh[:], sl, mx[:, 0:1], None,
                                            op0=OP.subtract)
                    ex = p6s.tile([1, S], f32, tag="ex", name="ex")
                    sm = p6s.tile([1, 1], f32, tag="sm", name="sm")
                    nc.scalar.activation(ex[:], sh[:], AF.Exp, accum_out=sm[:])
                    rc = p6s.tile([1, 1], f32, tag="rc", name="rc")
                    nc.vector.reciprocal(rc[:], sm[:])
                    nc.vector.tensor_scalar(wl, ex[:], rc[:, 0:1], None, op0=OP.mult)
                scores_w[kind] = w_
                if dbg and kind == "ds":
                    nc.sync.dma_start(dbg["d_wds"].ap(), w_[:].bitcast(f32))

        with tc.tile_pool(name="p6pb", bufs=1, space="PSUM") as p6pb:
            wbc = {}
            for kind in ("ds", "ts"):
                ps2 = [p6pb.tile([128, 512], f32, tag=f"wb{kind}{nh}",
                                 name=f"wb{kind}{nh}") for nh in range(2)]
                for nh in range(2):
                    nc.tensor.matmul(ps2[nh][:], onesr[:, 0:128],
                                     scores_w[kind][:, nh * 512:(nh + 1) * 512],
                                     start=True, stop=True)
                wbc[kind] = ps2
            for c in range(4):
                a = p6one.tile([128, SD], f32, tag="ctxa", name="ctxa")
                b_ = p6one.tile([128, SD], f32, tag="ctxb", name="ctxb")
                for nh in range(2):
                    nc.vector.tensor_mul(a[:, nh * 512:(nh + 1) * 512],
                                         docrepT[c][:, nh * 512:(nh + 1) * 512],
                                         wbc["ds"][nh][:])
                    nc.vector.tensor_mul(b_[:, nh * 512:(nh + 1) * 512],
                                         topicrepT[c][:, nh * 512:(nh + 1) * 512],
                                         wbc["ts"][nh][:])
                nc.vector.tensor_add(docrepT[c][:], a[:], b_[:])

        with tc.tile_pool(name="p6pd", bufs=4, space="PSUM") as p6pd, \
             tc.tile_pool(name="p6pl", bufs=1, space="PSUM") as p6pl:
            hdna = []
            for m2 in range(2):
                pm = [p6pd.tile([128, 512], f32, tag="dnap", name="dnap")
                      for _ in range(2)]
                for k in range(8):
                    for nh in range(2):
                        rhsap = (srepT[:, k * SD + nh * 512:k * SD + (nh + 1) * 512]
                                 if k < 4 else
                                 docrepT[k - 4][:, nh * 512:(nh + 1) * 512])
                        nc.tensor.matmul(pm[nh][:],
                                         wdna[k][:, m2 * 128:(m2 + 1) * 128],
                                         rhsap, start=(k == 0), stop=(k == 7))
                hd = p6one.tile([128, H4], f32r, tag=f"hdna{m2}", name=f"hdna{m2}")
                for nh in range(2):
                    nc.scalar.activation(hd[:, nh * 512:(nh + 1) * 512], pm[nh][:],
                                         AF.Relu, bias=bdna[:, m2:m2 + 1])
                hdna.append(hd)

            lg_ps = [p6pl.tile([1, 512], f32, tag=f"lgp{nh}", name=f"lgp{nh}")
                     for nh in range(2)]
            for k2 in range(2):
                for nh in range(2):
                    nc.tensor.matmul(lg_ps[nh][:], wout[:, k2:k2 + 1],
                                     hdna[k2][:, nh * 512:(nh + 1) * 512],
                                     start=(k2 == 0), stop=(k2 == 1))
            lg = p6one.tile([1, SD], f32, tag="lg", name="lg")
            for nh in range(2):
                nc.scalar.activation(lg[:, nh * 512:(nh + 1) * 512], lg_ps[nh][:],
                                     AF.Identity, bias=bout[:, 0:1])
            nc.sync.dma_start(logits.ap(), lg[:])

    ctx.close()


def _build():
    nc = bacc.Bacc("TRN2", target_bir_lowering=False, debug=False, num_devices=NC)
    ein = {}

    def inp(name, shape, dt=f32):
        ein[name] = nc.dram_tensor(name, shape, dt, kind="ExternalInput")

    inp("wid", [128, NBLK * L], i32)
    inp("tse", [BD, T, 2], i32)
    inp("emb", [V, E])
    inp("whhT_f", [H, H3], f32r)
    inp("whhT_b", [H, H3], f32r)
    inp("wihT_f", [E, H3], f32r)
    inp("wihT_b", [E, H3], f32r)
    inp("brow_f", [1, H3], f32r)
    inp("brow_b", [1, H3], f32r)
    inp("bhhn_f", [1, H], f32r)
    inp("bhhn_b", [1, H], f32r)
    inp("w_att", [H4, H4], f32r)
    inp("v_att", [H4, 1], f32r)
    inp("w_dna", [H4, D], f32r)
    inp("b_dna", [D])
    inp("w_out", [D, 1], f32r)
    inp("b_out", [1])
    inp("dvrows", [8, 1], i32)
    logits = nc.dram_tensor("logits", [1, SD], f32, kind="ExternalOutput")

    import os
    dbg = {}
    if int(os.environ.get("KDBG", "0")):
        for nm, shape in [("d_xT0", [128, SD]), ("d_gif", [128, GI]),
                          ("d_srepT", [128, 4 * SD]), ("d_trep0", [128, SD]),
                          ("d_wds", [1, SD])]:
            dbg[nm] = nc.dram_tensor(nm, shape, f32, kind="ExternalOutput")
    with tile.TileContext(nc) as tc:
        _emit(tc, nc, ein, logits, dbg)
    nc.compile()
    return nc


def _pack_core(c, word_ids, topic_start_ends, emb, Wih_f, Whh_f, bih_f, bhh_f,
               Wih_b, Whh_b, bih_b, bhh_b, W_att, v_att, W_dna, b_dna, W_out, b_out):
    w = word_ids[c * BD:(c + 1) * BD]                           # [BD, S, L]
    # wid[p, bl*L + l] = w[d, bl*32 + p//4, l], with p = (s%32)*4 + d
    arr = np.transpose(w, (1, 0, 2)).reshape(NBLK, 32, BD, L)   # [bl, s32, d, L]
    wid = np.ascontiguousarray(
        np.transpose(arr, (1, 2, 0, 3)).reshape(128, NBLK * L)).astype(np.int32)
    dvrows = np.zeros((8, 1), np.int32)
    for d in range(BD):
        b = c * BD + d
        if b < 16:
            g0, g1 = 2 * b, 2 * b + 1
            rows = ((g0 // BD) * 8 + g0 % BD, (g1 // BD) * 8 + g1 % BD)
        else:
            g0, g1 = 2 * b - 32, 2 * b + 1 - 32
            rows = ((g0 // BD) * 8 + BD + g0 % BD, (g1 // BD) * 8 + BD + g1 % BD)
        dvrows[2 * d, 0], dvrows[2 * d + 1, 0] = rows
    f32c = lambda x: np.ascontiguousarray(x, dtype=np.float32)

    def brow_of(bih, bhh):
        br = np.asarray(bih, np.float64).copy()
        br[:2 * H] += np.asarray(bhh, np.float64)[:2 * H]       # rz: bih+bhh; n: bih
        return f32c(br).reshape(1, H3)

    return {
        "wid": wid,
        "tse": np.ascontiguousarray(topic_start_ends[c * BD:(c + 1) * BD],
                                    dtype=np.int32),
        "emb": f32c(emb),
        "whhT_f": f32c(Whh_f.T), "whhT_b": f32c(Whh_b.T),
        "wihT_f": f32c(np.asarray(Wih_f, np.float64).T / L),
        "wihT_b": f32c(np.asarray(Wih_b, np.float64).T / L),
        "brow_f": brow_of(bih_f, bhh_f),
        "brow_b": brow_of(bih_b, bhh_b),
        "bhhn_f": f32c(bhh_f[2 * H:]).reshape(1, H),
        "bhhn_b": f32c(bhh_b[2 * H:]).reshape(1, H),
        "w_att": f32c(W_att), "v_att": f32c(v_att),
        "w_dna": f32c(W_dna), "b_dna": f32c(b_dna),
        "w_out": f32c(W_out), "b_out": f32c(b_out),
        "dvrows": dvrows,
    }


def kernel(**inputs):
    global _BUILT
    inputs = {k: np.asarray(v) for k, v in inputs.items()}
    if _BUILT is None:
        _BUILT = _build()
    nc = _BUILT
    in_maps = [_pack_core(c, **inputs) for c in range(NC)]
    res = run_bass_kernel_spmd(nc, in_maps, core_ids=list(range(NC)))
    out = np.zeros((B, S), np.float32)
    for c in range(NC):
        out[c * BD:(c + 1) * BD] = res.results[c]["logits"].reshape(S, BD).T
    return out
